# revision 1
# baseline (speedup 1.0000x reference)
"""Selective SSM (Mamba-1 style) layer on 8 Trainium2 NeuronCores.

Sharding: core c -> batch b = c // 2, d_model half dh = c % 2 (512 channels).
Each core is fully independent (scan recurrence is elementwise in d), so no
collectives are needed.

Per-core dataflow, layout [d on partitions, t on free]:
  PE : delta/B/C projections (bf16), identity-matmul PSUM accumulation of
       sum_n C_n * h_n, final y transpose back to [t, d].
  ACT: softplus(delta), exp(A[d,n] * dt[d,t]) via per-partition scale.
  DVE: u_n = dtx * B_rep_n (bf16 2x), tensor_tensor_scan recurrence (1x),
       prod_n = h_n * C_rep_n (bf16 2x).
  DMA: xbar-transposed loads of x, partition-broadcast of B/C rows.
"""

import numpy as np
import ml_dtypes
from contextlib import ExitStack

import concourse.bacc as bacc
import concourse.bass as bass
import concourse.mybir as mybir
import concourse.tile as tile
from concourse.bass_utils import run_bass_kernel_spmd

BF16 = ml_dtypes.bfloat16
F32 = mybir.dt.float32
B16 = mybir.dt.bfloat16

B_SZ, SEQ, D, N = 4, 2048, 1024, 16
DL = 512            # d_model channels per core
ND = DL // 128      # 4 d-tiles
NK = D // 128       # 8 contraction tiles
TB = SEQ // 512     # 4 moving-dim blocks for matmul
NT = SEQ // 128     # 16 t-tiles for output
NHALF = 2           # n-loop halves (SBUF pressure for B/C broadcasts)
NH = N // NHALF     # 8 states per half

_CACHE = {}


def _build():
    if "nc" in _CACHE:
        return _CACHE["nc"]
    mult = mybir.AluOpType.mult
    add = mybir.AluOpType.add

    nc = bacc.Bacc("TRN2", target_bir_lowering=False, debug=False, num_devices=8)

    xb16_d = nc.dram_tensor("xb16", [SEQ, D], B16, kind="ExternalInput")
    xsl16_d = nc.dram_tensor("xsl16", [SEQ, DL], B16, kind="ExternalInput")
    xd_d = nc.dram_tensor("xd", [SEQ, DL], F32, kind="ExternalInput")
    wd16_d = nc.dram_tensor("wd16", [D, DL], B16, kind="ExternalInput")
    wb16_d = nc.dram_tensor("wb16", [D, N], B16, kind="ExternalInput")
    wc16_d = nc.dram_tensor("wc16", [D, N], B16, kind="ExternalInput")
    aneg_d = nc.dram_tensor("aneg", [DL, N], F32, kind="ExternalInput")
    bd_d = nc.dram_tensor("bd", [DL, 1], F32, kind="ExternalInput")
    bb_d = nc.dram_tensor("bb", [N, 1], F32, kind="ExternalInput")
    bc_d = nc.dram_tensor("bc", [N, 1], F32, kind="ExternalInput")
    id16_d = nc.dram_tensor("id16", [128, 128], B16, kind="ExternalInput")
    id32_d = nc.dram_tensor("id32", [128, 128], F32, kind="ExternalInput")
    y_d = nc.dram_tensor("y", [SEQ, DL], F32, kind="ExternalOutput")

    with tile.TileContext(nc) as tc, ExitStack() as ctx:
        consts = ctx.enter_context(tc.tile_pool(name="consts", bufs=1))
        persist = ctx.enter_context(tc.tile_pool(name="persist", bufs=1))
        ps_mm = ctx.enter_context(tc.tile_pool(name="ps_mm", bufs=2, space="PSUM"))
        ps_y = ctx.enter_context(tc.tile_pool(name="ps_y", bufs=1, space="PSUM"))
        ps_t = ctx.enter_context(tc.tile_pool(name="ps_t", bufs=2, space="PSUM"))
        big16 = ctx.enter_context(tc.tile_pool(name="big16", bufs=16))
        work = ctx.enter_context(tc.tile_pool(name="work", bufs=2))
        outp = ctx.enter_context(tc.tile_pool(name="outp", bufs=2))

        # ---- constants / weights (coalesced: SBUF tiles pad to 4KB/part) ----
        wdall = consts.tile([128, NK * DL], B16, tag="wdall", name="wdall")
        wball = consts.tile([128, NK * N], B16, tag="wball", name="wball")
        wcall = consts.tile([128, NK * N], B16, tag="wcall", name="wcall")
        for k in range(NK):
            nc.sync.dma_start(wdall[:, k * DL:(k + 1) * DL],
                              wd16_d[k * 128:(k + 1) * 128, :])
            nc.sync.dma_start(wball[:, k * N:(k + 1) * N],
                              wb16_d[k * 128:(k + 1) * 128, :])
            nc.sync.dma_start(wcall[:, k * N:(k + 1) * N],
                              wc16_d[k * 128:(k + 1) * 128, :])
        wd_sb = [wdall[:, k * DL:(k + 1) * DL] for k in range(NK)]
        wb_sb = [wball[:, k * N:(k + 1) * N] for k in range(NK)]
        wc_sb = [wcall[:, k * N:(k + 1) * N] for k in range(NK)]
        abd = []
        for m in range(ND):
            t = consts.tile([128, N + 1], F32, tag=f"abd{m}", name=f"abd{m}")
            nc.sync.dma_start(t[:, 0:N], aneg_d[m * 128:(m + 1) * 128, :])
            nc.sync.dma_start(t[:, N:N + 1], bd_d[m * 128:(m + 1) * 128, :])
            abd.append(t)
        a_sb = [t[:, 0:N] for t in abd]
        bd_sb = [t[:, N:N + 1] for t in abd]
        bbc = consts.tile([N, 2], F32, tag="bbc", name="bbc")
        nc.sync.dma_start(bbc[:, 0:1], bb_d[:, :])
        nc.sync.dma_start(bbc[:, 1:2], bc_d[:, :])
        bb_sb = bbc[:, 0:1]
        bc_sb = bbc[:, 1:2]
        id16_sb = consts.tile([128, 128], B16, tag="id16", name="id16sb")
        nc.sync.dma_start(id16_sb[:], id16_d[:, :])
        id32_sb = consts.tile([128, 128], F32, tag="id32", name="id32sb")
        nc.sync.dma_start(id32_sb[:], id32_d[:, :])

        # ---- x^T via DMA xbar transpose ----
        xt = []
        for k in range(NK):
            t = big16.tile([128, SEQ], B16, tag="big16", name=f"xt{k}")
            nc.sync.dma_start_transpose(t[:], xb16_d[:, k * 128:(k + 1) * 128])
            xt.append(t)
        xts = []
        for m in range(ND):
            t = big16.tile([128, SEQ], B16, tag="big16", name=f"xts{m}")
            nc.sync.dma_start_transpose(t[:], xsl16_d[:, m * 128:(m + 1) * 128])
            xts.append(t)

        # ---- delta projection + softplus -> dt[m] (f32) ----
        dt_sb = [persist.tile([128, SEQ], B16, tag=f"dt{m}", name=f"dtv{m}") for m in range(ND)]
        for m in range(ND):
            for tb in range(TB):
                ps = ps_mm.tile([128, 512], F32, tag="mm", name="mmps")
                for k in range(NK):
                    nc.tensor.matmul(
                        ps[:], wd_sb[k][:, m * 128:(m + 1) * 128],
                        xt[k][:, tb * 512:(tb + 1) * 512],
                        start=(k == 0), stop=(k == NK - 1),
                    )
                et = outp.tile([128, 512], F32, tag="et", name="etv")
                nc.scalar.activation(
                    et[:], ps[:], mybir.ActivationFunctionType.Exp,
                    bias=bd_sb[m], scale=1.0,
                )
                nc.scalar.activation(
                    dt_sb[m][:, tb * 512:(tb + 1) * 512], et[:],
                    mybir.ActivationFunctionType.Ln, bias=1.0, scale=1.0,
                )

        # ---- B/C projections -> b16/c16 [N, SEQ] bf16 ----
        bmat = persist.tile([N, SEQ], B16, tag="bmat", name="bmat")
        cmat = persist.tile([N, SEQ], B16, tag="cmat", name="cmat")
        for tb in range(TB):
            psb = ps_mm.tile([N, 512], F32, tag="mm", name="mmpp")
            for k in range(NK):
                nc.tensor.matmul(
                    psb[:], wb_sb[k], xt[k][:, tb * 512:(tb + 1) * 512],
                    start=(k == 0), stop=(k == NK - 1),
                )
            nc.scalar.activation(
                bmat[:, tb * 512:(tb + 1) * 512], psb[:],
                mybir.ActivationFunctionType.Identity, bias=bb_sb, scale=1.0,
            )
            psc = ps_mm.tile([N, 512], F32, tag="mm", name="mmpp")
            for k in range(NK):
                nc.tensor.matmul(
                    psc[:], wc_sb[k], xt[k][:, tb * 512:(tb + 1) * 512],
                    start=(k == 0), stop=(k == NK - 1),
                )
            nc.scalar.activation(
                cmat[:, tb * 512:(tb + 1) * 512], psc[:],
                mybir.ActivationFunctionType.Identity, bias=bc_sb, scale=1.0,
            )

        # ---- dtx[m] = dt[m] * x^T[dsl] (bf16) ----
        dtx_sb = [persist.tile([128, SEQ], B16, tag=f"dtx{m}", name=f"dtx{m}") for m in range(ND)]
        for m in range(ND):
            nc.vector.tensor_mul(dtx_sb[m][:], dt_sb[m][:], xts[m][:])

        # ---- bounce B/C to DRAM for partition-broadcast reads ----
        dram = ctx.enter_context(tc.tile_pool(name="dram", bufs=1, space="DRAM"))
        bmat_dr = dram.tile([N, SEQ], B16, tag="bmat_dr", name="bmat_dr")
        cmat_dr = dram.tile([N, SEQ], B16, tag="cmat_dr", name="cmat_dr")
        nc.sync.dma_start(bmat_dr[:], bmat[:])
        nc.sync.dma_start(cmat_dr[:], cmat[:])

        # ---- scan phase ----
        y_sb = [persist.tile([128, SEQ], F32, tag=f"ysb{m}", name=f"ysb{m}") for m in range(ND)]
        for half in range(NHALF):
            breps = []
            creps = []
            for j in range(NH):
                n = half * NH + j
                br = big16.tile([128, SEQ], B16, tag="big16", name="brep")
                nc.sync.dma_start(br[:], bmat_dr[n:n + 1, :].partition_broadcast(128))
                breps.append(br)
                cr = big16.tile([128, SEQ], B16, tag="big16", name="crep")
                nc.sync.dma_start(cr[:], cmat_dr[n:n + 1, :].partition_broadcast(128))
                creps.append(cr)
            for m in range(ND):
                yps = ps_y.tile([128, SEQ], F32, tag="yps", name="ypsv")
                for j in range(NH):
                    n = half * NH + j
                    bar = work.tile([128, SEQ], F32, tag="bar", name="barv")
                    nc.scalar.activation(
                        bar[:], dt_sb[m][:],
                        mybir.ActivationFunctionType.Exp,
                        bias=0.0, scale=a_sb[m][:, n:n + 1],
                    )
                    u = work.tile([128, SEQ], B16, tag="u", name="uv")
                    nc.vector.tensor_mul(u[:], dtx_sb[m][:], breps[j][:])
                    h = work.tile([128, SEQ], B16, tag="h", name="hv")
                    nc.vector.tensor_tensor_scan(
                        h[:], bar[:], u[:], 0.0, op0=mult, op1=add,
                    )
                    prod = work.tile([128, SEQ], B16, tag="prod", name="prodv")
                    nc.vector.tensor_mul(prod[:], h[:], creps[j][:])
                    for tb in range(TB):
                        nc.tensor.matmul(
                            yps[:, tb * 512:(tb + 1) * 512], id16_sb[:],
                            prod[:, tb * 512:(tb + 1) * 512],
                            start=(j == 0), stop=(j == NH - 1),
                        )
                if half == 0:
                    nc.scalar.activation(
                        y_sb[m][:], yps[:],
                        mybir.ActivationFunctionType.Copy, bias=0.0, scale=1.0,
                    )
                else:
                    nc.vector.tensor_add(y_sb[m][:], y_sb[m][:], yps[:])

        # ---- transpose y back to [t, d], add skip, store ----
        for tt in range(NT):
            xdt = outp.tile([128, DL], F32, tag="xdt", name="xdtv")
            nc.sync.dma_start(xdt[:], xd_d[tt * 128:(tt + 1) * 128, :])
            yout = outp.tile([128, DL], F32, tag="yout", name="youtv")
            for m in range(ND):
                yt_ps = ps_t.tile([128, 128], F32, tag="ytp", name="ytpv")
                nc.tensor.transpose(
                    yt_ps[:], y_sb[m][:, tt * 128:(tt + 1) * 128], id32_sb[:],
                )
                nc.vector.tensor_add(
                    yout[:, m * 128:(m + 1) * 128],
                    xdt[:, m * 128:(m + 1) * 128], yt_ps[:],
                )
            nc.sync.dma_start(y_d[tt * 128:(tt + 1) * 128, :], yout[:])

    nc.compile()
    _CACHE["nc"] = nc
    return nc


def _in_maps(x, A_log, D_skip, Wd, bd, Wb, bb, Wc, bc):
    A = (-np.exp(np.asarray(A_log, np.float64))).astype(np.float32)
    x = np.asarray(x, np.float32)
    maps = []
    for c in range(8):
        b, dh = c // 2, c % 2
        dsl = slice(dh * DL, (dh + 1) * DL)
        maps.append({
            "xb16": x[b].astype(BF16),
            "xsl16": x[b][:, dsl].astype(BF16),
            "xd": (x[b][:, dsl] * np.asarray(D_skip)[None, dsl]).astype(np.float32),
            "wd16": np.asarray(Wd)[:, dsl].astype(BF16),
            "wb16": np.asarray(Wb).astype(BF16),
            "wc16": np.asarray(Wc).astype(BF16),
            "aneg": A[dsl],
            "bd": np.asarray(bd, np.float32)[dsl].reshape(DL, 1),
            "bb": np.asarray(bb, np.float32).reshape(N, 1),
            "bc": np.asarray(bc, np.float32).reshape(N, 1),
            "id16": np.eye(128, dtype=BF16),
            "id32": np.eye(128, dtype=np.float32),
        })
    return maps


def kernel(x, A_log, D_skip, Wd, bd, Wb, bb, Wc, bc, _trace=False):
    nc = _build()
    maps = _in_maps(x, A_log, D_skip, Wd, bd, Wb, bb, Wc, bc)
    res = run_bass_kernel_spmd(nc, maps, list(range(8)), trace=_trace)
    y = np.zeros((B_SZ, SEQ, D), np.float32)
    for c, om in enumerate(res.results):
        b, dh = c // 2, c % 2
        y[b][:, dh * DL:(dh + 1) * DL] = om["y"]
    if _trace:
        kernel.last_result = res
    return y

